# revision 7
# baseline (speedup 1.0000x reference)
"""Grid (voxel) mean-pooling kernel for Trainium2, 8 NeuronCores.

Algorithm
---------
reference: voxels = floor(x * 20); hash h = (v0*d1 + v1)*d2 + v2 after a
per-axis min shift; output row r = mean of points whose hash is the r-th
smallest distinct hash; rows >= n_unique are zero.

Device part (per core, data-parallel over point chunks):
  - 500k points / core, padded to 128 partitions x 3968 points, stored
    chunk-planar: [chunk][coord][column] so per-coordinate slices are
    contiguous.
  - floor via the round-to-nearest magic trick on (s - 0.5): exact for all
    non-integer s = 20*x; the measure-zero set where s is exactly an odd
    integer floors one too low on device and is corrected exactly on the
    host (expected ~10 coords out of 12M).
  - h = (v0*20 + v1)*20 + v2 in [0, 8000); split h = hi*128 + lo.
  - one-hot builds are ALL bf16 with a pair-packed innermost dim
    ([... , l, t2] with t2 = 2 tile-columns) so every DVE tensor_tensor
    qualifies for the 2x_1p perf mode (2 elem/cycle/lane); the hi one-hot
    and the tail of the lo one-hot run on the otherwise-idle Pool engine.
  - per 128-point tile: stationary one-hot(lo) (128x128 bf16), moving
    [onehot(hi) | f0*oh | f1*oh | f2*oh] (128x256 bf16); one PE matmul per
    tile accumulates into a single PSUM tile (128x256 f32) over all 3968
    tiles.
  - PSUM -> SBUF -> DRAM partial (128 x 256 f32) per core.

(walrus only gives TensorScalarPtr-style instructions a single sync-wait
slot, which Tile's multi-wait scheduling violates -> no tensor_scalar /
scalar_tensor_tensor anywhere; scalar*x+b runs on Act, everything else is
tensor_tensor with broadcast APs.)

Host part: sum the 8 partials, apply the odd-integer floor correction,
recover per-voxel counts and frac sums, remap device bins to the reference
hash order, mean = (v + sum_f/count) * 0.05.
"""

import sys

for p in ("/opt/trn_rl_repo",):
    if p not in sys.path:
        sys.path.insert(0, p)

import numpy as np
import ml_dtypes

P = 128
TPP = 3968          # points per partition per core (padded)
NPC = P * TPP       # 507904 >= 500000 points per core
N_CORES = 8
CHUNK = 128         # tile-columns per chunk
NCHUNK = TPP // CHUNK
NG = 2              # build groups per chunk
TPG = CHUNK // NG   # tile-columns per group (64)
GP = TPG // 2       # column-pairs per group (32)
HI = 64             # padded hi bins (63 used: h < 8000 -> hi <= 62)
LO = 128
NMOV = 4 * HI       # moving block width: counts | f0 | f1 | f2
MAGIC2 = float(1.5 * 2.0 ** 23)   # ulp-1 grid covers +-0.5 offsets safely
PAD_VAL = 2.0       # pad points hash out of range -> zero contribution
# walrus rejects TensorTensor is_equal on the Pool engine (no GPSIMD
# firmware); only add/mult lower there. Pool therefore takes the tail
# POOL_COLS hi-columns of each f-block multiply, everything else on DVE.
POOL_COLS = 54      # per f-block hi-columns multiplied on Pool (0..HI)

_CACHED = {}


def _build_bass():
    from concourse import mybir
    from concourse.bacc import Bacc
    from concourse.tile import TileContext

    f32 = mybir.dt.float32
    bf16 = mybir.dt.bfloat16
    Alu = mybir.AluOpType
    Act = mybir.ActivationFunctionType

    nc = Bacc("TRN2")
    x_in = nc.dram_tensor("x", (P, NCHUNK * 3 * CHUNK), f32,
                          kind="ExternalInput")
    il2_in = nc.dram_tensor("il2", (P, LO * 2), bf16, kind="ExternalInput")
    ih2_in = nc.dram_tensor("ih2", (P, HI * 2), bf16, kind="ExternalInput")
    out = nc.dram_tensor("partial", (P, NMOV), f32, kind="ExternalOutput")

    W = 3 * CHUNK
    n_tiles = NCHUNK * CHUNK

    with TileContext(nc) as tc:
        with (
            tc.tile_pool(name="const", bufs=1) as const_pool,
            tc.tile_pool(name="xin", bufs=3) as x_pool,
            tc.tile_pool(name="hash", bufs=2) as hash_pool,
            tc.tile_pool(name="oh", bufs=3) as oh_pool,
            tc.tile_pool(name="z", bufs=3) as z_pool,
            tc.tile_pool(name="res", bufs=1) as res_pool,
            tc.tile_pool(name="acc", bufs=1, space="PSUM") as psum_pool,
        ):
            il2 = const_pool.tile([P, LO * 2], bf16)
            nc.sync.dma_start(il2[:], il2_in[:, :])
            ih2 = const_pool.tile([P, HI * 2], bf16)
            nc.sync.dma_start(ih2[:], ih2_in[:, :])

            il2_v = il2[:].rearrange("p (l t) -> p l t", t=2)
            ih2_b = ih2[:].rearrange("p (b t) -> p b t", t=2) \
                .unsqueeze(1).to_broadcast([P, GP, HI, 2])

            acc = psum_pool.tile([P, NMOV], mybir.dt.float32)

            for ci in range(NCHUNK):
                xt = x_pool.tile([P, W], f32)
                nc.sync.dma_start(xt[:], x_in[:, ci * W:(ci + 1) * W])

                # s = 20*x; v = RNE((s - 0.5) + M2) - M2 == floor(s) a.s.
                s = hash_pool.tile([P, W], f32, tag="s")
                nc.scalar.activation(s[:], xt[:], Act.Copy, scale=20.0)
                s2 = hash_pool.tile([P, W], f32, tag="s2")
                nc.scalar.activation(s2[:], xt[:], Act.Copy, scale=20.0,
                                     bias=-0.5)
                r = hash_pool.tile([P, W], f32, tag="r")
                nc.scalar.activation(r[:], s2[:], Act.Copy, bias=MAGIC2)
                v = hash_pool.tile([P, W], f32, tag="v")
                nc.scalar.activation(v[:], r[:], Act.Copy, bias=-MAGIC2)
                fb = hash_pool.tile([P, W], bf16, tag="fb")
                nc.vector.tensor_tensor(fb[:], s[:], v[:], Alu.subtract)

                # h = (v0*20 + v1)*20 + v2   (planar slices)
                m1 = hash_pool.tile([P, CHUNK], f32, tag="m1")
                nc.scalar.activation(m1[:], v[:, 0:CHUNK], Act.Copy,
                                     scale=20.0)
                t1 = hash_pool.tile([P, CHUNK], f32, tag="t1")
                nc.vector.tensor_tensor(t1[:], m1[:], v[:, CHUNK:2 * CHUNK],
                                        Alu.add)
                m2 = hash_pool.tile([P, CHUNK], f32, tag="m2")
                nc.scalar.activation(m2[:], t1[:], Act.Copy, scale=20.0)
                h = hash_pool.tile([P, CHUNK], f32, tag="h")
                nc.vector.tensor_tensor(h[:], m2[:], v[:, 2 * CHUNK:W],
                                        Alu.add)

                # hi = floor(h/128) exactly: h/128 is exact with granularity
                # 2^-7, so biasing by -0.5 + 2^-9 never lands on a RNE tie
                # and always rounds to floor (incl. h = 128*m exactly).
                q2 = hash_pool.tile([P, CHUNK], f32, tag="q2")
                nc.scalar.activation(q2[:], h[:], Act.Copy,
                                     scale=1.0 / 128.0,
                                     bias=-0.5 + 2.0 ** -9)
                r2 = hash_pool.tile([P, CHUNK], f32, tag="r2")
                nc.scalar.activation(r2[:], q2[:], Act.Copy, bias=MAGIC2)
                hif = hash_pool.tile([P, CHUNK], f32, tag="hif")
                nc.scalar.activation(hif[:], r2[:], Act.Copy, bias=-MAGIC2)
                hm = hash_pool.tile([P, CHUNK], f32, tag="hm")
                nc.scalar.activation(hm[:], hif[:], Act.Copy, scale=-128.0)
                lof = hash_pool.tile([P, CHUNK], f32, tag="lof")
                nc.vector.tensor_tensor(lof[:], h[:], hm[:], Alu.add)

                lo_bf = hash_pool.tile([P, CHUNK], bf16, tag="lo_bf")
                nc.scalar.activation(lo_bf[:], lof[:], Act.Copy)
                hi_bf = hash_pool.tile([P, CHUNK], bf16, tag="hi_bf")
                nc.scalar.activation(hi_bf[:], hif[:], Act.Copy)

                for g in range(NG):
                    c0 = g * TPG
                    lo_g = lo_bf[:, c0:c0 + TPG] \
                        .rearrange("p (gp t) -> p gp t", t=2).unsqueeze(2)
                    hi_g = hi_bf[:, c0:c0 + TPG] \
                        .rearrange("p (gp t) -> p gp t", t=2).unsqueeze(2)

                    # one-hot(lo): [P, GP, LO, 2] on DVE (2x_1p)
                    olo = oh_pool.tile([P, GP * LO * 2], bf16)
                    olo_v = olo[:].rearrange("p (g l t) -> p g l t",
                                             l=LO, t=2)
                    nc.vector.tensor_tensor(
                        olo_v[:, :, :, :],
                        il2_v[:, :, :].unsqueeze(1)
                        .to_broadcast([P, GP, LO, 2]),
                        lo_g.to_broadcast([P, GP, LO, 2]),
                        Alu.is_equal)

                    # z = [oh | f0*oh | f1*oh | f2*oh]: [P, GP, 4, HI, 2]
                    z = z_pool.tile([P, GP * 4 * HI * 2], bf16)
                    z5 = z[:].rearrange("p (g k b t) -> p g k b t",
                                        k=4, b=HI, t=2)
                    oh_sl = z5[:, :, 0, :, :]
                    nc.vector.tensor_tensor(
                        oh_sl, ih2_b, hi_g.to_broadcast([P, GP, HI, 2]),
                        Alu.is_equal)
                    bd = HI - POOL_COLS
                    for c in range(3):
                        f_g = fb[:, c * CHUNK + c0:c * CHUNK + c0 + TPG] \
                            .rearrange("p (gp t) -> p gp t", t=2) \
                            .unsqueeze(2)
                        nc.vector.tensor_tensor(
                            z5[:, :, 1 + c, 0:bd, :],
                            z5[:, :, 0, 0:bd, :],
                            f_g.to_broadcast([P, GP, bd, 2]), Alu.mult)
                        if POOL_COLS:
                            nc.gpsimd.tensor_tensor(
                                z5[:, :, 1 + c, bd:HI, :],
                                z5[:, :, 0, bd:HI, :],
                                f_g.to_broadcast([P, GP, POOL_COLS, 2]),
                                Alu.mult)

                    for pi in range(GP):
                        for sl in range(2):
                            ti = ci * CHUNK + c0 + pi * 2 + sl
                            nc.tensor.matmul(
                                out=acc[:],
                                lhsT=olo_v[:, pi, :, sl],
                                rhs=z5[:, pi, :, :, sl],
                                start=(ti == 0),
                                stop=(ti == n_tiles - 1),
                            )

            res = res_pool.tile([P, NMOV], f32)
            nc.scalar.copy(res[:], acc[:])
            nc.sync.dma_start(out[:, :], res[:])

    nc.finalize()
    return nc


def _get_nc():
    if "nc" not in _CACHED:
        _CACHED["nc"] = _build_bass()
    return _CACHED["nc"]


def _make_in_maps(x: np.ndarray):
    N = x.shape[0]
    per_core = (N + N_CORES - 1) // N_CORES
    assert per_core <= NPC, (per_core, NPC)
    il2 = np.ascontiguousarray(np.broadcast_to(
        np.repeat(np.arange(LO, dtype=np.float32), 2), (P, LO * 2))
        .astype(ml_dtypes.bfloat16))
    ih2 = np.ascontiguousarray(np.broadcast_to(
        np.repeat(np.arange(HI, dtype=np.float32), 2), (P, HI * 2))
        .astype(ml_dtypes.bfloat16))
    in_maps = []
    for c in range(N_CORES):
        shard = x[c * per_core:(c + 1) * per_core]
        buf = np.full((NPC, 3), PAD_VAL, dtype=np.float32)
        buf[:shard.shape[0]] = shard
        # chunk-planar: [p, chunk, coord, col]
        xd = buf.reshape(P, NCHUNK, CHUNK, 3).transpose(0, 1, 3, 2)
        in_maps.append({
            "x": np.ascontiguousarray(xd).reshape(P, NCHUNK * 3 * CHUNK),
            "il2": il2,
            "ih2": ih2,
        })
    return in_maps


def _apply_floor_fix(x, cnt2, fs):
    """Fix points whose s = 20*x hits an exact odd integer: the device's
    RNE(s-0.5) floors them one too low. Exact, O(#flagged) work."""
    s = x * np.float32(20.0)                        # matches device Act mult
    rs = np.round(s)
    bad = (s == rs) & (rs.astype(np.int64) % 2 == 1)
    rows = np.nonzero(bad.any(axis=1))[0]
    if len(rows) == 0:
        return
    bf = ml_dtypes.bfloat16
    for i in rows:
        v_true = np.floor(s[i].astype(np.float64)).astype(np.int64)
        v_dev = v_true - bad[i].astype(np.int64)     # device floored 1 low
        f_dev = (s[i] - v_dev.astype(np.float32)).astype(bf)
        f_true = (s[i] - v_true.astype(np.float32)).astype(bf)
        hd = (v_dev[0] * 20 + v_dev[1]) * 20 + v_dev[2]
        ht = (v_true[0] * 20 + v_true[1]) * 20 + v_true[2]
        for hh, ff, sgn in ((hd, f_dev, -1.0), (ht, f_true, +1.0)):
            lo_i, hi_i = int(hh) % 128, int(hh) // 128
            cnt2[lo_i, hi_i] += sgn
            for d in range(3):
                fs[d][lo_i, hi_i] += sgn * float(ff[d])


def kernel(x: np.ndarray) -> np.ndarray:
    from concourse import bass_utils

    x = np.ascontiguousarray(x, dtype=np.float32)
    N = x.shape[0]
    assert x.shape == (N, 3)

    # host-side metadata pass (cheap): exact same f32 voxelization as the
    # device computes, used only for min/dims/bin-order remapping.
    v_host = np.floor(x * np.float32(20.0)).astype(np.int64)
    vmin = v_host.min(axis=0)
    vmax = v_host.max(axis=0)
    assert (vmin >= 0).all() and (vmax <= 19).all(), (vmin, vmax)
    dims = vmax - vmin + 1

    nc = _get_nc()
    res = bass_utils.run_bass_kernel_spmd(
        nc, _make_in_maps(x), core_ids=list(range(N_CORES)))
    agg = np.zeros((P, NMOV), dtype=np.float64)
    for m in res.results:
        agg += m["partial"].astype(np.float64)

    # agg[lo, blk*HI + hi]: blk 0 = counts, 1..3 = frac sums
    cnt2 = agg[:, 0:HI]          # [lo, hi]
    fs = [agg[:, (k + 1) * HI:(k + 2) * HI] for k in range(3)]
    _apply_floor_fix(x, cnt2, fs)

    hbins = np.arange(8000)
    lo_i = hbins % 128
    hi_i = hbins // 128
    counts = cnt2[lo_i, hi_i]                      # per device-bin h
    present = counts > 0.5

    v0 = hbins // 400
    v1 = (hbins // 20) % 20
    v2 = hbins % 20
    # reference hash with data-derived min/dims (a.s. identical to h itself)
    ref_hash = ((v0 - vmin[0]) * dims[1] + (v1 - vmin[1])) * dims[2] \
        + (v2 - vmin[2])

    out = np.zeros((N, 3), dtype=np.float32)
    pres_idx = np.nonzero(present)[0]
    order = np.argsort(ref_hash[pres_idx], kind="stable")
    src = pres_idx[order]                          # device bins in uniq order
    cnts = counts[src]
    vs = np.stack([v0[src], v1[src], v2[src]], axis=1).astype(np.float64)
    fsum = np.stack([fs[k][lo_i[src], hi_i[src]] for k in range(3)], axis=1)
    means = (vs + fsum / cnts[:, None]) * 0.05
    out[:len(src)] = means.astype(np.float32)
    return out


if __name__ == "__main__":
    rng = np.random.default_rng(0)
    x = rng.random((200000, 3), dtype=np.float32)
    o = kernel(x)
    print(o.shape, o.dtype, o[:3])


# revision 19
# speedup vs baseline: 1.7616x; 1.7616x over previous
"""Grid (voxel) mean-pooling kernel for Trainium2, 8 NeuronCores.

Algorithm
---------
reference: voxels = floor(x * 20); hash h = (v0*d1 + v1)*d2 + v2 after a
per-axis min shift; output row r = mean of points whose hash is the r-th
smallest distinct hash; rows >= n_unique are zero.

Device part (per core, data-parallel over point chunks):
  - 500k points / core, padded to 128 partitions x 3968 points, stored
    chunk-planar: [chunk][coord][column] so per-coordinate slices are
    contiguous.
  - floor via the round-to-nearest magic trick on (s - 0.5): exact for all
    non-integer s = 20*x; the measure-zero set where s is exactly an odd
    integer floors one too low on device and is corrected exactly on the
    host (expected ~10 coords out of 12M).
  - h = (v0*20 + v1)*20 + v2 in [0, 8000); split h = hi*128 + lo.
  - one-hot builds are ALL bf16 with a pair-packed innermost dim
    ([... , l, t2] with t2 = 2 tile-columns) so every DVE tensor_tensor
    qualifies for the 2x_1p perf mode (2 elem/cycle/lane); the hi one-hot
    and the tail of the lo one-hot run on the otherwise-idle Pool engine.
  - per 128-point tile: stationary one-hot(lo) (128x128 bf16), moving
    [onehot(hi) | f0*oh | f1*oh | f2*oh] (128x256 bf16); one PE matmul per
    tile accumulates into a single PSUM tile (128x256 f32) over all 3968
    tiles.
  - PSUM -> SBUF -> DRAM partial (128 x 256 f32) per core.

(walrus only gives TensorScalarPtr-style instructions a single sync-wait
slot, which Tile's multi-wait scheduling violates -> no tensor_scalar /
scalar_tensor_tensor anywhere; scalar*x+b runs on Act, everything else is
tensor_tensor with broadcast APs.)

Host part: sum the 8 partials, apply the odd-integer floor correction,
recover per-voxel counts and frac sums, remap device bins to the reference
hash order, mean = (v + sum_f/count) * 0.05.
"""

import sys

for p in ("/opt/trn_rl_repo",):
    if p not in sys.path:
        sys.path.insert(0, p)

import numpy as np
import ml_dtypes

P = 128
TPP = 3968          # points per partition per core (padded)
NPC = P * TPP       # 507904 >= 500000 points per core
N_CORES = 8
CHUNK = 128         # tile-columns per chunk
NCHUNK = TPP // CHUNK
NG = 2              # build groups per chunk
TPG = CHUNK // NG   # tile-columns per group (64)
GP = TPG // 2       # column-pairs per group (32)
HI = 64             # padded hi bins (63 used: h < 8000 -> hi <= 62)
LO = 128
NMOV = 4 * HI       # moving block width: counts | f0 | f1 | f2
MAGIC2 = float(1.5 * 2.0 ** 23)   # ulp-1 grid covers +-0.5 offsets safely
PAD_VAL = 2.0       # pad points hash out of range -> zero contribution
# walrus rejects TensorTensor is_equal on the Pool engine (no GPSIMD
# firmware); only add/mult lower there. Pool therefore takes the tail
# POOL_COLS hi-columns of each f-block multiply, everything else on DVE.
POOL_COLS = 0       # per f-block hi-columns multiplied on Pool (0..HI);
                    # 0: real GPSIMD tensor_tensor costs ~2.5us/instr in
                    # kernel context, which loses more than it offloads.
NREP = 1            # timing harness: repeat the whole pass via tc.For_i
MODE = "full"       # timing harness: "full" | "nomm" (no matmuls) |
                    # "nobuild" (matmuls on constant tiles)

_CACHED = {}


def _build_bass():
    from concourse import mybir
    from concourse.bacc import Bacc
    from concourse.tile import TileContext

    f32 = mybir.dt.float32
    bf16 = mybir.dt.bfloat16
    Alu = mybir.AluOpType
    Act = mybir.ActivationFunctionType

    nc = Bacc("TRN2")
    x_in = nc.dram_tensor("x", (P, NCHUNK * 3 * CHUNK), f32,
                          kind="ExternalInput")
    il2_in = nc.dram_tensor("il2", (P, LO * 2), bf16, kind="ExternalInput")
    # ih2 pre-repeated per pair-group so the oh build has a single
    # broadcast operand (the hi comparand)
    ih2_in = nc.dram_tensor("ih2", (P, GP * HI * 2), bf16,
                            kind="ExternalInput")
    out = nc.dram_tensor("partial", (P, NMOV), f32, kind="ExternalOutput")

    W = 3 * CHUNK
    n_tiles = NCHUNK * CHUNK

    with TileContext(nc) as tc:
        with (
            tc.tile_pool(name="const", bufs=1) as const_pool,
            tc.tile_pool(name="xin", bufs=3) as x_pool,
            tc.tile_pool(name="hash", bufs=2) as hash_pool,
            tc.tile_pool(name="oh", bufs=3) as oh_pool,
            tc.tile_pool(name="z", bufs=3) as z_pool,
            tc.tile_pool(name="res", bufs=1) as res_pool,
            tc.tile_pool(name="acc", bufs=1, space="PSUM") as psum_pool,
        ):
            il2 = const_pool.tile([P, LO * 2], bf16)
            nc.sync.dma_start(il2[:], il2_in[:, :])
            ih2 = const_pool.tile([P, GP * HI * 2], bf16)
            nc.sync.dma_start(ih2[:], ih2_in[:, :])

            il2_v = il2[:].rearrange("p (l t) -> p l t", t=2)
            ih2_b = ih2[:].rearrange("p (g b t) -> p g b t", b=HI, t=2)

            acc = psum_pool.tile([P, NMOV], mybir.dt.float32)

            if MODE == "nobuild":
                olo_c = const_pool.tile([P, GP * LO * 2], bf16)
                nc.vector.memset(olo_c[:], 0.0)
                z_c = const_pool.tile([P, GP * 4 * HI * 2], bf16)
                nc.vector.memset(z_c[:], 0.0)
                olo_cv = olo_c[:].rearrange("p (g l t) -> p g l t",
                                            l=LO, t=2)
                z_cv = z_c[:].rearrange("p (g k b t) -> p g k b t",
                                        k=4, b=HI, t=2)
                n_total = NCHUNK * CHUNK
                for ti in range(n_total):
                    pi, sl = (ti // 2) % GP, ti % 2
                    nc.tensor.matmul(
                        out=acc[:], lhsT=olo_cv[:, pi, :, sl],
                        rhs=z_cv[:, pi, :, :, sl],
                        start=(ti == 0), stop=(ti == n_total - 1))

            import contextlib
            loop_cm = tc.For_i(0, NREP) if NREP > 1 \
                else contextlib.nullcontext()
            with loop_cm:
              for ci in range(NCHUNK if MODE != "nobuild" else 0):
                xt = x_pool.tile([P, W], f32)
                nc.sync.dma_start(xt[:], x_in[:, ci * W:(ci + 1) * W])

                # s = 20*x; v = RNE((s - 0.5) + M2) - M2 == floor(s) a.s.
                s = hash_pool.tile([P, W], f32, tag="s")
                nc.scalar.activation(s[:], xt[:], Act.Copy, scale=20.0)
                s2 = hash_pool.tile([P, W], f32, tag="s2")
                nc.scalar.activation(s2[:], xt[:], Act.Copy, scale=20.0,
                                     bias=-0.5)
                r = hash_pool.tile([P, W], f32, tag="r")
                nc.scalar.activation(r[:], s2[:], Act.Copy, bias=MAGIC2)
                v = hash_pool.tile([P, W], f32, tag="v")
                nc.scalar.activation(v[:], r[:], Act.Copy, bias=-MAGIC2)
                fb = hash_pool.tile([P, W], bf16, tag="fb")
                nc.vector.tensor_tensor(fb[:], s[:], v[:], Alu.subtract)

                # h = (v0*20 + v1)*20 + v2   (planar slices)
                m1 = hash_pool.tile([P, CHUNK], f32, tag="m1")
                nc.scalar.activation(m1[:], v[:, 0:CHUNK], Act.Copy,
                                     scale=20.0)
                t1 = hash_pool.tile([P, CHUNK], f32, tag="t1")
                nc.vector.tensor_tensor(t1[:], m1[:], v[:, CHUNK:2 * CHUNK],
                                        Alu.add)
                m2 = hash_pool.tile([P, CHUNK], f32, tag="m2")
                nc.scalar.activation(m2[:], t1[:], Act.Copy, scale=20.0)
                h = hash_pool.tile([P, CHUNK], f32, tag="h")
                nc.vector.tensor_tensor(h[:], m2[:], v[:, 2 * CHUNK:W],
                                        Alu.add)

                # hi = floor(h/128) exactly: h/128 is exact with granularity
                # 2^-7, so biasing by -0.5 + 2^-9 never lands on a RNE tie
                # and always rounds to floor (incl. h = 128*m exactly).
                q2 = hash_pool.tile([P, CHUNK], f32, tag="q2")
                nc.scalar.activation(q2[:], h[:], Act.Copy,
                                     scale=1.0 / 128.0,
                                     bias=-0.5 + 2.0 ** -9)
                r2 = hash_pool.tile([P, CHUNK], f32, tag="r2")
                nc.scalar.activation(r2[:], q2[:], Act.Copy, bias=MAGIC2)
                hif = hash_pool.tile([P, CHUNK], f32, tag="hif")
                nc.scalar.activation(hif[:], r2[:], Act.Copy, bias=-MAGIC2)
                hm = hash_pool.tile([P, CHUNK], f32, tag="hm")
                nc.scalar.activation(hm[:], hif[:], Act.Copy, scale=-128.0)
                lof = hash_pool.tile([P, CHUNK], f32, tag="lof")
                nc.vector.tensor_tensor(lof[:], h[:], hm[:], Alu.add)

                lo_bf = hash_pool.tile([P, CHUNK], bf16, tag="lo_bf")
                nc.scalar.activation(lo_bf[:], lof[:], Act.Copy)
                hi_bf = hash_pool.tile([P, CHUNK], bf16, tag="hi_bf")
                nc.scalar.activation(hi_bf[:], hif[:], Act.Copy)

                for g in range(NG):
                    c0 = g * TPG
                    lo_g = lo_bf[:, c0:c0 + TPG] \
                        .rearrange("p (gp t) -> p gp t", t=2).unsqueeze(2)
                    hi_g = hi_bf[:, c0:c0 + TPG] \
                        .rearrange("p (gp t) -> p gp t", t=2).unsqueeze(2)

                    # one-hot(lo): [P, GP, LO, 2] on DVE (2x_1p)
                    olo = oh_pool.tile([P, GP * LO * 2], bf16)
                    olo_v = olo[:].rearrange("p (g l t) -> p g l t",
                                             l=LO, t=2)
                    nc.vector.tensor_tensor(
                        olo_v[:, :, :, :],
                        il2_v[:, :, :].unsqueeze(1)
                        .to_broadcast([P, GP, LO, 2]),
                        lo_g.to_broadcast([P, GP, LO, 2]),
                        Alu.is_equal)

                    # z = [oh | f0*oh | f1*oh | f2*oh]: [P, GP, 4, HI, 2]
                    z = z_pool.tile([P, GP * 4 * HI * 2], bf16)
                    z5 = z[:].rearrange("p (g k b t) -> p g k b t",
                                        k=4, b=HI, t=2)
                    oh_sl = z5[:, :, 0, :, :]
                    nc.vector.tensor_tensor(
                        oh_sl, ih2_b, hi_g.to_broadcast([P, GP, HI, 2]),
                        Alu.is_equal)
                    bd = HI - POOL_COLS
                    for c in range(3):
                        f_g = fb[:, c * CHUNK + c0:c * CHUNK + c0 + TPG] \
                            .rearrange("p (gp t) -> p gp t", t=2) \
                            .unsqueeze(2)
                        nc.vector.tensor_tensor(
                            z5[:, :, 1 + c, 0:bd, :],
                            z5[:, :, 0, 0:bd, :],
                            f_g.to_broadcast([P, GP, bd, 2]), Alu.mult)
                        if POOL_COLS:
                            nc.gpsimd.tensor_tensor(
                                z5[:, :, 1 + c, bd:HI, :],
                                z5[:, :, 0, bd:HI, :],
                                f_g.to_broadcast([P, GP, POOL_COLS, 2]),
                                Alu.mult)

                    for pi in range(GP):
                        for sl in range(2):
                            ti = ci * CHUNK + c0 + pi * 2 + sl
                            if MODE == "nomm" and ti not in (
                                    0, n_tiles - 1):
                                continue  # builds-only timing variant
                            nc.tensor.matmul(
                                out=acc[:],
                                lhsT=olo_v[:, pi, :, sl],
                                rhs=z5[:, pi, :, :, sl],
                                start=(ti == 0),
                                stop=(ti == n_tiles - 1),
                            )

            res = res_pool.tile([P, NMOV], f32)
            nc.scalar.copy(res[:], acc[:])
            nc.sync.dma_start(out[:, :], res[:])

    nc.finalize()
    return nc


def _get_nc():
    if "nc" not in _CACHED:
        _CACHED["nc"] = _build_bass()
    return _CACHED["nc"]


def _make_in_maps(x: np.ndarray):
    N = x.shape[0]
    per_core = (N + N_CORES - 1) // N_CORES
    assert per_core <= NPC, (per_core, NPC)
    il2 = np.ascontiguousarray(np.broadcast_to(
        np.repeat(np.arange(LO, dtype=np.float32), 2), (P, LO * 2))
        .astype(ml_dtypes.bfloat16))
    ih2 = np.ascontiguousarray(np.broadcast_to(
        np.tile(np.repeat(np.arange(HI, dtype=np.float32), 2), GP),
        (P, GP * HI * 2)).astype(ml_dtypes.bfloat16))
    in_maps = []
    for c in range(N_CORES):
        shard = x[c * per_core:(c + 1) * per_core]
        buf = np.full((NPC, 3), PAD_VAL, dtype=np.float32)
        buf[:shard.shape[0]] = shard
        # chunk-planar: [p, chunk, coord, col]
        xd = buf.reshape(P, NCHUNK, CHUNK, 3).transpose(0, 1, 3, 2)
        in_maps.append({
            "x": np.ascontiguousarray(xd).reshape(P, NCHUNK * 3 * CHUNK),
            "il2": il2,
            "ih2": ih2,
        })
    return in_maps


def _apply_floor_fix(x, cnt2, fs):
    """Fix points whose s = 20*x hits an exact odd integer: the device's
    RNE(s-0.5) floors them one too low. Exact, O(#flagged) work."""
    s = x * np.float32(20.0)                        # matches device Act mult
    rs = np.round(s)
    bad = (s == rs) & (rs.astype(np.int64) % 2 == 1)
    rows = np.nonzero(bad.any(axis=1))[0]
    if len(rows) == 0:
        return
    bf = ml_dtypes.bfloat16
    for i in rows:
        v_true = np.floor(s[i].astype(np.float64)).astype(np.int64)
        v_dev = v_true - bad[i].astype(np.int64)     # device floored 1 low
        f_dev = (s[i] - v_dev.astype(np.float32)).astype(bf)
        f_true = (s[i] - v_true.astype(np.float32)).astype(bf)
        hd = (v_dev[0] * 20 + v_dev[1]) * 20 + v_dev[2]
        ht = (v_true[0] * 20 + v_true[1]) * 20 + v_true[2]
        for hh, ff, sgn in ((hd, f_dev, -1.0), (ht, f_true, +1.0)):
            lo_i, hi_i = int(hh) % 128, int(hh) // 128
            cnt2[lo_i, hi_i] += sgn
            for d in range(3):
                fs[d][lo_i, hi_i] += sgn * float(ff[d])


def kernel(x: np.ndarray) -> np.ndarray:
    from concourse import bass_utils

    x = np.ascontiguousarray(x, dtype=np.float32)
    N = x.shape[0]
    assert x.shape == (N, 3)

    # host-side metadata pass (cheap): exact same f32 voxelization as the
    # device computes, used only for min/dims/bin-order remapping.
    v_host = np.floor(x * np.float32(20.0)).astype(np.int64)
    vmin = v_host.min(axis=0)
    vmax = v_host.max(axis=0)
    assert (vmin >= 0).all() and (vmax <= 19).all(), (vmin, vmax)
    dims = vmax - vmin + 1

    nc = _get_nc()
    res = bass_utils.run_bass_kernel_spmd(
        nc, _make_in_maps(x), core_ids=list(range(N_CORES)))
    agg = np.zeros((P, NMOV), dtype=np.float64)
    for m in res.results:
        agg += m["partial"].astype(np.float64)

    # agg[lo, blk*HI + hi]: blk 0 = counts, 1..3 = frac sums
    cnt2 = agg[:, 0:HI]          # [lo, hi]
    fs = [agg[:, (k + 1) * HI:(k + 2) * HI] for k in range(3)]
    _apply_floor_fix(x, cnt2, fs)

    hbins = np.arange(8000)
    lo_i = hbins % 128
    hi_i = hbins // 128
    counts = cnt2[lo_i, hi_i]                      # per device-bin h
    present = counts > 0.5

    v0 = hbins // 400
    v1 = (hbins // 20) % 20
    v2 = hbins % 20
    # reference hash with data-derived min/dims (a.s. identical to h itself)
    ref_hash = ((v0 - vmin[0]) * dims[1] + (v1 - vmin[1])) * dims[2] \
        + (v2 - vmin[2])

    out = np.zeros((N, 3), dtype=np.float32)
    pres_idx = np.nonzero(present)[0]
    order = np.argsort(ref_hash[pres_idx], kind="stable")
    src = pres_idx[order]                          # device bins in uniq order
    cnts = counts[src]
    vs = np.stack([v0[src], v1[src], v2[src]], axis=1).astype(np.float64)
    fsum = np.stack([fs[k][lo_i[src], hi_i[src]] for k in range(3)], axis=1)
    means = (vs + fsum / cnts[:, None]) * 0.05
    out[:len(src)] = means.astype(np.float32)
    return out


if __name__ == "__main__":
    rng = np.random.default_rng(0)
    x = rng.random((200000, 3), dtype=np.float32)
    o = kernel(x)
    print(o.shape, o.dtype, o[:3])
